# revision 22
# baseline (speedup 1.0000x reference)
"""EquivariantGNN message-passing kernel for Trainium2 (8 NeuronCores, SPMD).

Math (matches the reference):
  x   = [pos | onehot(z)] @ [[I3,0],[0,emb]]          (rank-8 node features)
  logits[e,h] = 0.25 * q[dst]. (k[src]+eb)
              = sum_{i,j} x8[dst][i]*srcext[e][j] * Bvec[(i,j),h]
  w = exp(logits)  (no max subtraction needed; logits are O(10))
  den[n,h] = sum_{dst(e)=n} w
  U[n,(j,h)] = sum_{dst(e)=n} w[e,h]*srcext[e,j]      (96 values per node)
  agg[n,h,:] = (U[n,:,h]/den[n,h]) @ Wve12[:,h-slice]  (ve folded per NODE)
  out = agg @ Wo + x ; S = sum_n relu(out) ; answer = (S @ lin_w)/N + lin_b

Device strategy per core: edges sorted by dst, 128-edge blocks each fully inside
one 128-node tile.  Host ships, per edge, the 96-dim kron row
x8[dst] (x) srcext in fp8 (lhsT layout — fp8 halves the dominant DMA stream
and is validated to ~3e-4 final error), srcext row-major, and the
onehot(localdst) fp8 scatter matrix.  Per block one tiny matmul (rhs [96,8])
yields the logits; ACT exponentiates into the payload tile [P, GB, 104]
(cols = [w(8) | (j,h)(96)]); a DVE tensor_tensor forms w (x) srcext; the
onehot fp8 matmul scatter-accumulates (contiguous rhs — a strided rhs runs
the PE moving-fetch at half speed, measured).  Ghost edges cover every node
slot so den>0 by construction and the epilogue needs no +eps guard: plain
reciprocal, a 3D scalar_tensor_tensor normalize (innermost-packed rden
broadcast), PE transpose, one (WblkWo|residual) matmul, relu, and a
ones-matmul accumulates S.
"""

import math
import os
import sys

import numpy as np

for _p in ("/opt/trn_rl_repo", "/root/.axon_site/_ro/trn_rl_repo"):
    if os.path.isdir(_p) and _p not in sys.path:
        sys.path.insert(0, _p)

P = 128
DIM = 128
H, DH = 8, 16
DE = 4
NF12 = 12   # srcext = [ea(4) | x8[src](8)]
NKRON = 96  # logit lhsT rows: kron of x8dst (8) and srcext (12)
NU = 8 + NKRON  # scatter payload: [w(8) | w (x) srcext (96) in (j,h) order]
N_CORES = 8
GB = 8       # blocks per processing group
GCHUNK = 32  # blocks per gather chunk

# test-harness knobs (the grading harness just calls kernel() with defaults)
PROFILE = False
TRACE_CORES = None
LAST_RESULT = None  # BassKernelResults of the last run (for profiling)
_PROG_CACHE = {}


# ---------------------------------------------------------------- host prep
def _host_prep(pos, edge_attr, emb, Wq, Wk, Wv, We, Wo, z, edge_index):
    f32 = np.float32
    N = pos.shape[0]
    NT = emb.shape[0]
    ntiles = (N + P - 1) // P
    npad = ntiles * P

    z = np.asarray(z).astype(np.int64)
    src = np.asarray(edge_index[0]).astype(np.int64)
    dst = np.asarray(edge_index[1]).astype(np.int64)
    E = src.shape[0]

    onehot = np.zeros((N, NT), f32)
    onehot[np.arange(N), z] = 1.0
    x8 = np.concatenate([np.asarray(pos, f32), onehot], axis=1)  # [N, 8]
    x8p = np.zeros((npad, 8), f32)
    x8p[:N] = x8

    # rank-8 weight factors
    Wq8 = np.vstack([Wq[:3], emb @ Wq[3:]]).astype(f32)  # [8,128]
    Wk8 = np.vstack([Wk[:3], emb @ Wk[3:]]).astype(f32)
    Wv8 = np.vstack([Wv[:3], emb @ Wv[3:]]).astype(f32)
    # srcext rows = [ea(4); x8src(8)]:  ke = srcext @ [[We],[Wk8]]
    Wke12 = np.vstack([We, Wk8]).astype(f32)   # [12,128]
    Wve12 = np.vstack([We, Wv8]).astype(f32)   # [12,128]

    # bilinear logits: logits[e,h] = sum_{i,j} x8dst[i]*srcext[j]*Bvec[(i,j),h]
    Bvec = np.zeros((NKRON, H), f32)
    for h in range(H):
        Bh = 0.25 * (Wq8[:, h * DH:(h + 1) * DH]
                     @ Wke12[:, h * DH:(h + 1) * DH].T)  # [8,12]
        Bvec[:, h] = Bh.reshape(NKRON)

    # U[(j,h)] -> out:  WblkWo[(j,h), d'] = sum_d Wve12[j, h*16+d] * Wo[h*16+d, d']
    Wo32 = np.asarray(Wo, f32)
    WblkWo = np.zeros((NKRON, DIM), f32)  # rows in (j,h) order: row = j*8+h
    for h in range(H):
        blk = Wve12[:, h * DH:(h + 1) * DH] @ Wo32[h * DH:(h + 1) * DH]  # [12,128]
        WblkWo[h::H] = blk

    J8 = np.zeros((8, DIM), f32)  # x = x8 @ J8 (residual)
    J8[0:3, 0:3] = np.eye(3, dtype=f32)
    J8[3:8, 3:DIM] = emb

    # ---- sort edges by dst, split into per-node-tile runs
    perm = np.argsort(dst, kind="stable")
    src_s, dst_s = src[perm], dst[perm]
    ea_s = np.asarray(edge_attr, f32)[perm]
    tile_of_edge = dst_s // P
    starts = np.searchsorted(tile_of_edge, np.arange(ntiles))
    ends = np.searchsorted(tile_of_edge, np.arange(ntiles) + 1)
    ecnt = ends - starts

    # ghost edges: every tile-local node slot with zero in-tile edges gets a
    # ghost (kron=0 -> w=1, sext=0, oh=onehot(slot)) so den>0 everywhere and
    # the epilogue needs no +eps guard.  Count them first to size the blocks.
    ncov = np.zeros(ntiles, np.int64)
    for t in range(ntiles):
        ncov[t] = np.unique(dst_s[starts[t]:ends[t]]).shape[0]
    nghost = P - ncov
    need = ecnt + nghost
    nb = np.maximum(1, (need + P - 1) // P)  # blocks per real tile

    # per-edge srcext + kron rows
    sext = np.empty((E, NF12), f32)
    sext[:, 0:DE] = ea_s
    sext[:, DE:NF12] = x8[src_s]
    kron = (x8[dst_s][:, :, None] * sext[:, None, :]).reshape(E, NKRON)

    # ---- uniform schedule across cores: pad tile list to multiple of 8,
    # sort by block count desc, deal groups of 8 (one tile per core),
    # pad each group to the group max -> identical counts on every core.
    ntiles_tot = ((ntiles + N_CORES - 1) // N_CORES) * N_CORES
    nb_all = np.concatenate([nb, np.ones(ntiles_tot - ntiles, np.int64)])
    order = np.argsort(-nb_all, kind="stable")
    TS = ntiles_tot // N_CORES  # tiles per core
    counts = [int(nb_all[order[8 * k]]) for k in range(TS)]  # group max
    counts[-1] += (-sum(counts)) % GB  # block count multiple of the group size
    C = int(sum(counts))

    import ml_dtypes

    bf16 = ml_dtypes.bfloat16
    fp8 = ml_dtypes.float8_e4m3fn

    srcfac = np.zeros((N_CORES, C, P, NKRON), fp8)
    se12t = np.zeros((N_CORES, P, C, NF12), bf16)
    ohmat = np.zeros((N_CORES, C, P, P), fp8)       # onehot(localdst)
    xT8c = np.zeros((N_CORES, 8, TS * P), f32)

    offs = np.concatenate([[0], np.cumsum(counts)])
    for k in range(TS):
        for j in range(N_CORES):
            t = int(order[8 * k + j])
            c0 = int(offs[k])
            if t >= ntiles:
                # dummy tile: one ghost per node slot so den=1 (first block
                # becomes the identity); remaining blocks stay all-dummy.
                ohmat[j, c0, np.arange(P), np.arange(P)] = 1.0
                continue
            xT8c[j, :, k * P:(k + 1) * P] = x8p[t * P:(t + 1) * P].T
            e0, e1 = int(starts[t]), int(ends[t])
            ne = e1 - e0
            loc = dst_s[e0:e1] - t * P
            covered = np.zeros(P, bool)
            covered[loc] = True
            ghosts = np.nonzero(~covered)[0]
            tot = ne + ghosts.shape[0]
            flat = np.arange(tot)
            cc = c0 + flat // P
            pp = flat % P
            if ne:
                srcfac[j, cc[:ne], pp[:ne], :] = kron[e0:e1]
                se12t[j, pp[:ne], cc[:ne], :] = sext[e0:e1]
                ohmat[j, cc[:ne], pp[:ne], loc] = 1.0
            if ghosts.shape[0]:
                ohmat[j, cc[ne:], pp[ne:], ghosts] = 1.0

    ones = np.ones((P, 1), f32)

    # device layouts
    srcfacT = np.ascontiguousarray(
        srcfac.transpose(0, 3, 1, 2)).reshape(N_CORES, NKRON, C * P)
    ohmatd = np.ascontiguousarray(ohmat.transpose(0, 2, 1, 3))  # [j, P, C, P]
    SE_SPLIT = min(56, C)
    se12a = np.ascontiguousarray(se12t[:, :, 0:SE_SPLIT, :])
    se12b = np.ascontiguousarray(se12t[:, :, SE_SPLIT:, :])

    WblkWoJ = np.vstack([WblkWo, J8])  # residual folded as 8 extra lhsT rows
    shared = dict(rhs96=Bvec.astype(bf16),
                  WblkWo=WblkWoJ.astype(bf16),
                  ident=np.eye(P, dtype=f32),
                  ones=ones.astype(bf16))
    percore = dict(srcfacT=srcfacT, se12a=se12a, se12b=se12b, ohmat=ohmatd,
                   xT8c=xT8c.astype(bf16))
    meta = dict(counts=counts, C=C, TS=TS, npad=npad, N=N, E=E,
                SE_SPLIT=SE_SPLIT)
    return shared, percore, meta


# ---------------------------------------------------------------- device code
def _build_program(counts, C, TS, npad, SE_SPLIT):
    import concourse.bacc as bacc
    import concourse.bass as bass
    import concourse.tile as tile
    from concourse import mybir
    from concourse._compat import with_exitstack  # noqa: F401

    f32 = mybir.dt.float32
    bf16 = mybir.dt.bfloat16
    fp8 = mybir.dt.float8e4

    nc = bacc.Bacc("TRN2", target_bir_lowering=False, debug=False,
                   enable_asserts=False, num_devices=N_CORES)

    srcfacT_in = nc.dram_tensor("srcfacT", [NKRON, C * P], fp8,
                                kind="ExternalInput").ap()
    se12a_in = nc.dram_tensor("se12a", [P, SE_SPLIT, NF12], bf16,
                              kind="ExternalInput").ap()
    se12b_in = nc.dram_tensor("se12b", [P, C - SE_SPLIT, NF12], bf16,
                              kind="ExternalInput").ap()
    ohmat_in = nc.dram_tensor("ohmat", [P, C, P], fp8, kind="ExternalInput").ap()
    xT8c_in = nc.dram_tensor("xT8c", [8, TS * P], bf16, kind="ExternalInput").ap()
    rhs96_in = nc.dram_tensor("rhs96", [NKRON, H], bf16,
                              kind="ExternalInput").ap()
    WblkWo_in = nc.dram_tensor("WblkWo", [NKRON + 8, DIM], bf16,
                               kind="ExternalInput").ap()
    ident_in = nc.dram_tensor("ident", [P, P], f32, kind="ExternalInput").ap()
    ones_in = nc.dram_tensor("ones", [P, 1], bf16, kind="ExternalInput").ap()
    S_out = nc.dram_tensor("S_out", [1, 4 * DIM], f32, kind="ExternalOutput").ap()

    with tile.TileContext(nc) as tc:
        with (
            tc.tile_pool(name="const", bufs=1) as constp,
            tc.tile_pool(name="chunks", bufs=6) as chunkp,
            tc.tile_pool(name="blk", bufs=6) as blkp,
            tc.tile_pool(name="psmain", bufs=2, space="PSUM") as psmainp,
            tc.tile_pool(name="psmisc", bufs=1, space="PSUM") as psmiscp,
            tc.tile_pool(name="psacc", bufs=3, space="PSUM") as psaccp,
            tc.tile_pool(name="psS", bufs=1, space="PSUM") as psSp,
        ):
            # HAM warmup: ~5us of back-to-back matmuls on a memset tile (no
            # DMA dependency) so the PE clock un-throttles to 2.4 GHz while
            # the first chunks stream in.
            warm_sb = constp.tile([P, 2 * P], bf16, tag="warmsrc")
            nc.gpsimd.memset(warm_sb[:], 0.5)
            pswarm = psmiscp.tile([P, DIM], f32, tag="T")
            for _ in range(16):
                nc.tensor.matmul(pswarm[:], lhsT=warm_sb[:, 0:P],
                                 rhs=warm_sb[:, P:2 * P], start=True, stop=True)

            # chunk schedule: small prologue chunks so compute starts early
            bounds = [0]
            for nxt in (8, 24, 56):
                if nxt < C:
                    bounds.append(nxt)
            while bounds[-1] + GCHUNK < C:
                bounds.append(bounds[-1] + GCHUNK)
            bounds.append(C)
            cidx_of = {}
            for ci in range(len(bounds) - 1):
                for g in range(bounds[ci], bounds[ci + 1]):
                    cidx_of[g] = ci

            chunks = {}

            def load_chunk(ci):
                g0, g1 = bounds[ci], bounds[ci + 1]
                gn = g1 - g0
                st = chunkp.tile([NKRON, GCHUNK * P], fp8, tag="srcT")
                ohc = chunkp.tile([P, GCHUNK, P], fp8, tag="ohc")
                nc.sync.dma_start(out=st[:, :gn * P],
                                  in_=srcfacT_in[:, g0 * P:(g0 + gn) * P])
                nc.sync.dma_start(out=ohc[:, :gn, :],
                                  in_=ohmat_in[:, g0:g0 + gn, :])
                chunks[ci] = (st, ohc, g0)

            # critical-path loads first: the logit weights, srcext rows and
            # the first two edge chunks; remaining constants go out on the
            # (otherwise idle) GpSimd DMA queue.
            load_chunk(0)
            rhs96_sb = constp.tile_from(rhs96_in)
            se12a_sb = constp.tile([P, SE_SPLIT, NF12], bf16, tag="se12a")
            nc.sync.dma_start(out=se12a_sb[:], in_=se12a_in)
            load_chunk(1)
            if len(bounds) > 3:
                load_chunk(2)
            se12b_sb = constp.tile([P, C - SE_SPLIT, NF12], bf16, tag="se12b")
            nc.sync.dma_start(out=se12b_sb[:], in_=se12b_in)
            pool_eng = mybir.EngineType.Pool
            WblkWo_sb = constp.tile_from(WblkWo_in, forced_dma_engine=pool_eng)
            xT8c_sb = constp.tile_from(xT8c_in, forced_dma_engine=pool_eng)
            identb_sb = constp.tile_from(ident_in, dtype=mybir.dt.bfloat16,
                                         forced_dma_engine=pool_eng,
                                         force_copy=True)
            ones_sb = constp.tile_from(ones_in, forced_dma_engine=pool_eng)

            psS = psSp.tile([1, 4 * DIM], f32, tag="S")
            hall = constp.tile([P, TS * DIM], bf16, tag="hall")
            nslice = (TS + 3) // 4

            # per-tile aggT lives in the const pool with the residual rows
            # (xT8c) pre-filled by GpSimd during the DMA ramp, so the
            # epilogue chain no longer waits on a 900ns Q7 copy
            aggTs = []
            for t in range(TS):
                at = constp.tile([NKRON + 8, P], bf16, tag=f"aggT{t}")
                nc.gpsimd.tensor_copy(at[NKRON:NKRON + 8, :],
                                      xT8c_sb[:, t * P:(t + 1) * P])
                aggTs.append(at)

            # block -> (tile, b, nb) map for the flat pair loop
            blk2tile = []
            for t in range(TS):
                for b in range(counts[t]):
                    blk2tile.append((t, b, counts[t]))

            def _epilogue(t, acc):
                # ghost edges cover every node slot, so den >= 1 there and
                # exp() never rounds to zero in bf16: no +eps guard needed
                rden = blkp.tile([P, H], f32, tag="rden")
                nc.vector.reciprocal(rden[:], acc[:, 0:8])
                aggs = blkp.tile([P, NKRON], bf16, tag="aggs")
                nc.vector.scalar_tensor_tensor(
                    out=aggs[:].rearrange("p (a b) -> p a b", b=H),
                    in0=acc[:, 8:NU].rearrange("p (a b) -> p a b", b=H),
                    scalar=1.0,
                    in1=rden[:, None, :].to_broadcast([P, NF12, H]),
                    op0=mybir.AluOpType.mult,
                    op1=mybir.AluOpType.mult,
                )
                psT2 = psmiscp.tile([NKRON, P], bf16, tag="T2")
                nc.tensor.transpose(out=psT2[:], in_=aggs[:], identity=identb_sb[:])
                aggT = aggTs[t]
                nc.scalar.copy(aggT[0:NKRON, :], psT2[:])
                pso = psmiscp.tile([P, DIM], f32, tag="T")
                nc.tensor.matmul(pso[:], lhsT=aggT[:], rhs=WblkWo_sb[:],
                                 start=True, stop=True)
                nc.scalar.activation(hall[:, t * DIM:(t + 1) * DIM], pso[:],
                                     mybir.ActivationFunctionType.Relu)
                if t % 4 == 3 or t == TS - 1:
                    s = t // 4
                    c0, c1 = s * 4 * DIM, (t + 1) * DIM
                    nc.tensor.matmul(psS[:, 0:c1 - c0], lhsT=ones_sb[:],
                                     rhs=hall[:, c0:c1],
                                     start=(s == 0), stop=(s == nslice - 1))

            acc_state = [None]

            def scatter_one(item, q):
                g0, rhswm, ohc, cb0 = item
                t, b, nb = blk2tile[g0 + q]
                if b == 0:
                    acc_state[0] = psaccp.tile([P, NU], f32, tag="acc",
                                               name="acc")
                acc = acc_state[0]
                nc.tensor.matmul(acc[:], lhsT=ohc[:, cb0 + q, :],
                                 rhs=rhswm[:, q, :],
                                 start=(b == 0), stop=(b == nb - 1))
                if b == nb - 1:
                    _epilogue(t, acc)

            pend = []
            for g in range(0, C, GB):
                ci = cidx_of[g]
                if g == bounds[ci] and ci not in chunks:
                    load_chunk(ci)
                st, ohc, cg0 = chunks[ci]
                cb = g - cg0

                # interleave this group's main matmuls 1:1 with the scatter
                # matmuls of the group issued two iterations ago: alternating
                # PSUM targets lets the PE pull the next weight-load ahead.
                sc = pend.pop(0) if len(pend) > 2 else None
                psm = psmainp.tile([P, GB, H], f32, tag="main")
                for q in range(GB):
                    nc.tensor.matmul(psm[:, q, :],
                                     lhsT=st[:, (cb + q) * P:(cb + q + 1) * P],
                                     rhs=rhs96_sb[:], start=True, stop=True)
                    if sc is not None:
                        scatter_one(sc, q)

                # payload cols = [w(8) | (j,h)(96)] so the epilogue's rden
                # broadcast is innermost-packed (3D scalar_tensor_tensor)
                rhswm = blkp.tile([P, GB, NU], bf16, tag="rhswm")
                nc.scalar.activation(rhswm[:, :, 0:8], psm[:],
                                     mybir.ActivationFunctionType.Exp)
                se_sb = se12a_sb if g + GB <= SE_SPLIT else se12b_sb
                gg = g if g + GB <= SE_SPLIT else g - SE_SPLIT
                nc.vector.tensor_tensor(
                    out=rhswm[:, :, 8:NU].rearrange("p c (a b) -> p c a b",
                                                    b=H),
                    in0=se_sb[:, gg:gg + GB, :, None].to_broadcast(
                        [P, GB, NF12, H]),
                    in1=rhswm[:, :, None, 0:8].to_broadcast([P, GB, NF12, H]),
                    op=mybir.AluOpType.mult,
                )
                pend.append((g, rhswm, ohc, cb))
            while pend:
                item = pend.pop(0)
                for q in range(GB):
                    scatter_one(item, q)

            Scopy = constp.tile([1, 4 * DIM], f32, tag="Scopy")
            nc.vector.tensor_copy(Scopy[:], psS[:])
            nc.sync.dma_start(out=S_out, in_=Scopy[:])

    nc.compile()
    return nc


# ---------------------------------------------------------------- entry point
def kernel(**inputs):
    pos = np.asarray(inputs["pos"], np.float32)
    edge_attr = np.asarray(inputs["edge_attr"], np.float32)
    emb = np.asarray(inputs["emb"], np.float32)
    Wq = np.asarray(inputs["Wq"], np.float32)
    Wk = np.asarray(inputs["Wk"], np.float32)
    Wv = np.asarray(inputs["Wv"], np.float32)
    We = np.asarray(inputs["We"], np.float32)
    Wo = np.asarray(inputs["Wo"], np.float32)
    lin_w = np.asarray(inputs["lin_w"], np.float32)
    lin_b = np.asarray(inputs["lin_b"], np.float32)
    z = inputs["z"]
    edge_index = inputs["edge_index"]

    shared, percore, meta = _host_prep(pos, edge_attr, emb, Wq, Wk, Wv, We, Wo,
                                       z, edge_index)
    N = meta["N"]

    key = (tuple(meta["counts"]), meta["C"], meta["TS"], meta["npad"],
           meta["SE_SPLIT"])
    nc = _PROG_CACHE.get(key)
    if nc is None:
        nc = _build_program(meta["counts"], meta["C"], meta["TS"],
                            meta["npad"], meta["SE_SPLIT"])
        _PROG_CACHE[key] = nc

    in_maps = []
    for j in range(N_CORES):
        m = {
            "rhs96": shared["rhs96"],
            "WblkWo": shared["WblkWo"],
            "ident": shared["ident"],
            "ones": shared["ones"],
            "srcfacT": percore["srcfacT"][j],
            "se12a": percore["se12a"][j],
            "se12b": percore["se12b"][j],
            "ohmat": percore["ohmat"][j],
            "xT8c": percore["xT8c"][j],
        }
        in_maps.append(m)

    from concourse.bass_utils import run_bass_kernel_spmd
    res = run_bass_kernel_spmd(nc, in_maps, core_ids=list(range(N_CORES)),
                               trace=PROFILE, trace_cores=TRACE_CORES)
    global LAST_RESULT
    LAST_RESULT = res
    S = np.zeros(DIM, np.float64)
    for r in res.results:
        S += r["S_out"][0].astype(np.float64).reshape(4, DIM).sum(axis=0)
    y = (S.astype(np.float32) @ lin_w) / np.float32(N) + lin_b
    return y.reshape(1, 1).astype(np.float32)


# revision 24
# speedup vs baseline: 1.4737x; 1.4737x over previous
"""EquivariantGNN message-passing kernel for Trainium2 (8 NeuronCores, SPMD).

Math (matches the reference):
  x   = [pos | onehot(z)] @ [[I3,0],[0,emb]]          (rank-8 node features)
  logits[e,h] = 0.25 * q[dst]. (k[src]+eb)
              = sum_{i,j} x8[dst][i]*srcext[e][j] * Bvec[(i,j),h]
  w = exp(logits)  (no max subtraction needed; logits are O(10))
  den[n,h] = sum_{dst(e)=n} w
  U[n,(j,h)] = sum_{dst(e)=n} w[e,h]*srcext[e,j]      (96 values per node)
  agg[n,h,:] = (U[n,:,h]/den[n,h]) @ Wve12[:,h-slice]  (ve folded per NODE)
  out = agg @ Wo + x ; S = sum_n relu(out) ; answer = (S @ lin_w)/N + lin_b

Device strategy per core: edges sorted by dst, 128-edge blocks each fully inside
one 128-node tile.  Host ships, per edge, the 96-dim kron row
x8[dst] (x) srcext in fp8 (lhsT layout — fp8 halves the dominant DMA stream
and is validated to ~3e-4 final error), srcext row-major, and the
onehot(localdst) fp8 scatter matrix.  Per block one tiny matmul (rhs [96,8])
yields the logits; ACT exponentiates into the payload tile [P, GB, 104]
(cols = [w(8) | (j,h)(96)]); a DVE tensor_tensor forms w (x) srcext; the
onehot fp8 matmul scatter-accumulates (contiguous rhs — a strided rhs runs
the PE moving-fetch at half speed, measured).  Ghost edges cover every node
slot so den>0 by construction and the epilogue needs no +eps guard: plain
reciprocal, a 3D scalar_tensor_tensor normalize (innermost-packed rden
broadcast), PE transpose, one (WblkWo|residual) matmul, relu, and a
ones-matmul accumulates S.
"""

import math
import os
import sys

import numpy as np

for _p in ("/opt/trn_rl_repo", "/root/.axon_site/_ro/trn_rl_repo"):
    if os.path.isdir(_p) and _p not in sys.path:
        sys.path.insert(0, _p)

P = 128
DIM = 128
H, DH = 8, 16
DE = 4
NF12 = 12   # srcext = [ea(4) | x8[src](8)]
NKRON = 96  # logit lhsT rows: kron of x8dst (8) and srcext (12)
NU = 8 + NKRON  # scatter payload: [w(8) | w (x) srcext (96) in (j,h) order]
N_CORES = 8
GB = 8       # blocks per processing group
GCHUNK = 32  # blocks per gather chunk

# test-harness knobs (the grading harness just calls kernel() with defaults)
PROFILE = False
TRACE_CORES = None
LAST_RESULT = None  # BassKernelResults of the last run (for profiling)
_PROG_CACHE = {}


# ---------------------------------------------------------------- host prep
def _host_prep(pos, edge_attr, emb, Wq, Wk, Wv, We, Wo, z, edge_index):
    f32 = np.float32
    N = pos.shape[0]
    NT = emb.shape[0]
    ntiles = (N + P - 1) // P
    npad = ntiles * P

    z = np.asarray(z).astype(np.int64)
    src = np.asarray(edge_index[0]).astype(np.int64)
    dst = np.asarray(edge_index[1]).astype(np.int64)
    E = src.shape[0]

    onehot = np.zeros((N, NT), f32)
    onehot[np.arange(N), z] = 1.0
    x8 = np.concatenate([np.asarray(pos, f32), onehot], axis=1)  # [N, 8]
    x8p = np.zeros((npad, 8), f32)
    x8p[:N] = x8

    # rank-8 weight factors
    Wq8 = np.vstack([Wq[:3], emb @ Wq[3:]]).astype(f32)  # [8,128]
    Wk8 = np.vstack([Wk[:3], emb @ Wk[3:]]).astype(f32)
    Wv8 = np.vstack([Wv[:3], emb @ Wv[3:]]).astype(f32)
    # srcext rows = [ea(4); x8src(8)]:  ke = srcext @ [[We],[Wk8]]
    Wke12 = np.vstack([We, Wk8]).astype(f32)   # [12,128]
    Wve12 = np.vstack([We, Wv8]).astype(f32)   # [12,128]

    # bilinear logits: logits[e,h] = sum_{i,j} x8dst[i]*srcext[j]*Bvec[(i,j),h]
    Bvec = np.zeros((NKRON, H), f32)
    for h in range(H):
        Bh = 0.25 * (Wq8[:, h * DH:(h + 1) * DH]
                     @ Wke12[:, h * DH:(h + 1) * DH].T)  # [8,12]
        Bvec[:, h] = Bh.reshape(NKRON)

    # U[(j,h)] -> out:  WblkWo[(j,h), d'] = sum_d Wve12[j, h*16+d] * Wo[h*16+d, d']
    Wo32 = np.asarray(Wo, f32)
    WblkWo = np.zeros((NKRON, DIM), f32)  # rows in (j,h) order: row = j*8+h
    for h in range(H):
        blk = Wve12[:, h * DH:(h + 1) * DH] @ Wo32[h * DH:(h + 1) * DH]  # [12,128]
        WblkWo[h::H] = blk

    J8 = np.zeros((8, DIM), f32)  # x = x8 @ J8 (residual)
    J8[0:3, 0:3] = np.eye(3, dtype=f32)
    J8[3:8, 3:DIM] = emb

    # ---- sort edges by dst, split into per-node-tile runs
    perm = np.argsort(dst, kind="stable")
    src_s, dst_s = src[perm], dst[perm]
    ea_s = np.asarray(edge_attr, f32)[perm]
    tile_of_edge = dst_s // P
    starts = np.searchsorted(tile_of_edge, np.arange(ntiles))
    ends = np.searchsorted(tile_of_edge, np.arange(ntiles) + 1)
    ecnt = ends - starts

    # ghost edges: every tile-local node slot with zero in-tile edges gets a
    # ghost (kron=0 -> w=1, sext=0, oh=onehot(slot)) so den>0 everywhere and
    # the epilogue needs no +eps guard.  Count them first to size the blocks.
    ncov = np.zeros(ntiles, np.int64)
    for t in range(ntiles):
        ncov[t] = np.unique(dst_s[starts[t]:ends[t]]).shape[0]
    nghost = P - ncov
    need = ecnt + nghost
    nb = np.maximum(1, (need + P - 1) // P)  # blocks per real tile

    # per-edge srcext + kron rows
    sext = np.empty((E, NF12), f32)
    sext[:, 0:DE] = ea_s
    sext[:, DE:NF12] = x8[src_s]
    kron = (x8[dst_s][:, :, None] * sext[:, None, :]).reshape(E, NKRON)

    # ---- uniform schedule across cores: pad tile list to multiple of 8,
    # sort by block count desc, deal groups of 8 (one tile per core),
    # pad each group to the group max -> identical counts on every core.
    ntiles_tot = ((ntiles + N_CORES - 1) // N_CORES) * N_CORES
    nb_all = np.concatenate([nb, np.ones(ntiles_tot - ntiles, np.int64)])
    order = np.argsort(-nb_all, kind="stable")
    TS = ntiles_tot // N_CORES  # tiles per core
    counts = [int(nb_all[order[8 * k]]) for k in range(TS)]  # group max
    counts[-1] += (-sum(counts)) % GB  # block count multiple of the group size
    C = int(sum(counts))

    import ml_dtypes

    bf16 = ml_dtypes.bfloat16
    fp8 = ml_dtypes.float8_e4m3fn

    srcfac = np.zeros((N_CORES, C, P, NKRON), fp8)
    se12t = np.zeros((N_CORES, P, C, NF12), bf16)
    ohmat = np.zeros((N_CORES, C, P, P), fp8)       # onehot(localdst)
    xT8c = np.zeros((N_CORES, 8, TS * P), f32)

    offs = np.concatenate([[0], np.cumsum(counts)])
    for k in range(TS):
        for j in range(N_CORES):
            t = int(order[8 * k + j])
            c0 = int(offs[k])
            if t >= ntiles:
                # dummy tile: one ghost per node slot so den=1 (first block
                # becomes the identity); remaining blocks stay all-dummy.
                ohmat[j, c0, np.arange(P), np.arange(P)] = 1.0
                continue
            xT8c[j, :, k * P:(k + 1) * P] = x8p[t * P:(t + 1) * P].T
            e0, e1 = int(starts[t]), int(ends[t])
            ne = e1 - e0
            loc = dst_s[e0:e1] - t * P
            covered = np.zeros(P, bool)
            covered[loc] = True
            ghosts = np.nonzero(~covered)[0]
            tot = ne + ghosts.shape[0]
            flat = np.arange(tot)
            cc = c0 + flat // P
            pp = flat % P
            if ne:
                srcfac[j, cc[:ne], pp[:ne], :] = kron[e0:e1]
                se12t[j, pp[:ne], cc[:ne], :] = sext[e0:e1]
                ohmat[j, cc[:ne], pp[:ne], loc] = 1.0
            if ghosts.shape[0]:
                ohmat[j, cc[ne:], pp[ne:], ghosts] = 1.0

    ones = np.ones((P, 1), f32)

    # device layouts
    srcfacT = np.ascontiguousarray(
        srcfac.transpose(0, 3, 1, 2)).reshape(N_CORES, NKRON, C * P)
    ohmatd = np.ascontiguousarray(ohmat.transpose(0, 2, 1, 3))  # [j, P, C, P]
    SE_SPLIT = min(56, C)
    se12a = np.ascontiguousarray(se12t[:, :, 0:SE_SPLIT, :])
    se12b = np.ascontiguousarray(se12t[:, :, SE_SPLIT:, :])

    WblkWoJ = np.vstack([WblkWo, J8])  # residual folded as 8 extra lhsT rows
    shared = dict(rhs96=Bvec.astype(bf16),
                  WblkWo=WblkWoJ.astype(bf16),
                  ident=np.eye(P, dtype=f32),
                  ones=ones.astype(bf16))
    percore = dict(srcfacT=srcfacT, se12a=se12a, se12b=se12b, ohmat=ohmatd,
                   xT8c=xT8c.astype(bf16))
    meta = dict(counts=counts, C=C, TS=TS, npad=npad, N=N, E=E,
                SE_SPLIT=SE_SPLIT)
    return shared, percore, meta


# ---------------------------------------------------------------- device code
def _build_program(counts, C, TS, npad, SE_SPLIT):
    import concourse.bacc as bacc
    import concourse.bass as bass
    import concourse.tile as tile
    from concourse import mybir
    from concourse._compat import with_exitstack  # noqa: F401

    f32 = mybir.dt.float32
    bf16 = mybir.dt.bfloat16
    fp8 = mybir.dt.float8e4

    nc = bacc.Bacc("TRN2", target_bir_lowering=False, debug=False,
                   enable_asserts=False, num_devices=N_CORES)

    srcfacT_in = nc.dram_tensor("srcfacT", [NKRON, C * P], fp8,
                                kind="ExternalInput").ap()
    se12a_in = nc.dram_tensor("se12a", [P, SE_SPLIT, NF12], bf16,
                              kind="ExternalInput").ap()
    se12b_in = nc.dram_tensor("se12b", [P, C - SE_SPLIT, NF12], bf16,
                              kind="ExternalInput").ap()
    ohmat_in = nc.dram_tensor("ohmat", [P, C, P], fp8, kind="ExternalInput").ap()
    xT8c_in = nc.dram_tensor("xT8c", [8, TS * P], bf16, kind="ExternalInput").ap()
    rhs96_in = nc.dram_tensor("rhs96", [NKRON, H], bf16,
                              kind="ExternalInput").ap()
    WblkWo_in = nc.dram_tensor("WblkWo", [NKRON + 8, DIM], bf16,
                               kind="ExternalInput").ap()
    ident_in = nc.dram_tensor("ident", [P, P], f32, kind="ExternalInput").ap()
    ones_in = nc.dram_tensor("ones", [P, 1], bf16, kind="ExternalInput").ap()
    S_out = nc.dram_tensor("S_out", [1, 4 * DIM], f32, kind="ExternalOutput").ap()

    with tile.TileContext(nc) as tc:
        with (
            tc.tile_pool(name="const", bufs=1) as constp,
            tc.tile_pool(name="chunks", bufs=6) as chunkp,
            tc.tile_pool(name="blk", bufs=6) as blkp,
            tc.tile_pool(name="psmain", bufs=2, space="PSUM") as psmainp,
            tc.tile_pool(name="psmisc", bufs=1, space="PSUM") as psmiscp,
            tc.tile_pool(name="psacc", bufs=3, space="PSUM") as psaccp,
            tc.tile_pool(name="psS", bufs=1, space="PSUM") as psSp,
        ):
            # HAM warmup: ~5us of back-to-back matmuls on a memset tile (no
            # DMA dependency) so the PE clock un-throttles to 2.4 GHz while
            # the first chunks stream in.
            warm_sb = constp.tile([P, 2 * P], bf16, tag="warmsrc")
            nc.gpsimd.memset(warm_sb[:], 0.5)
            pswarm = psmiscp.tile([P, DIM], f32, tag="T")
            for _ in range(16):
                nc.tensor.matmul(pswarm[:], lhsT=warm_sb[:, 0:P],
                                 rhs=warm_sb[:, P:2 * P], start=True, stop=True)

            # chunk schedule: small prologue chunks so compute starts early
            bounds = [0]
            for nxt in (8, 24, 56):
                if nxt < C:
                    bounds.append(nxt)
            while bounds[-1] + GCHUNK < C:
                bounds.append(bounds[-1] + GCHUNK)
            bounds.append(C)
            cidx_of = {}
            for ci in range(len(bounds) - 1):
                for g in range(bounds[ci], bounds[ci + 1]):
                    cidx_of[g] = ci

            chunks = {}

            def load_chunk(ci):
                g0, g1 = bounds[ci], bounds[ci + 1]
                gn = g1 - g0
                st = chunkp.tile([NKRON, GCHUNK * P], fp8, tag="srcT")
                ohc = chunkp.tile([P, GCHUNK, P], fp8, tag="ohc")
                nc.sync.dma_start(out=st[:, :gn * P],
                                  in_=srcfacT_in[:, g0 * P:(g0 + gn) * P])
                nc.sync.dma_start(out=ohc[:, :gn, :],
                                  in_=ohmat_in[:, g0:g0 + gn, :])
                chunks[ci] = (st, ohc, g0)

            # critical-path loads first: the logit weights, srcext rows and
            # the first two edge chunks; remaining constants go out on the
            # (otherwise idle) GpSimd DMA queue.
            load_chunk(0)
            rhs96_sb = constp.tile_from(rhs96_in)
            se12a_sb = constp.tile([P, SE_SPLIT, NF12], bf16, tag="se12a")
            nc.sync.dma_start(out=se12a_sb[:], in_=se12a_in)
            load_chunk(1)
            if len(bounds) > 3:
                load_chunk(2)
            se12b_sb = constp.tile([P, C - SE_SPLIT, NF12], bf16, tag="se12b")
            nc.sync.dma_start(out=se12b_sb[:], in_=se12b_in)
            pool_eng = mybir.EngineType.Pool
            WblkWo_sb = constp.tile_from(WblkWo_in, forced_dma_engine=pool_eng)
            xT8c_sb = constp.tile_from(xT8c_in, forced_dma_engine=pool_eng)
            identb_sb = constp.tile_from(ident_in, dtype=mybir.dt.bfloat16,
                                         forced_dma_engine=pool_eng,
                                         force_copy=True)
            ones_sb = constp.tile_from(ones_in, forced_dma_engine=pool_eng)

            psS = psSp.tile([1, 4 * DIM], f32, tag="S")
            hall = constp.tile([P, TS * DIM], bf16, tag="hall")
            nslice = (TS + 3) // 4

            # per-tile aggT lives in the const pool with the residual rows
            # (xT8c) pre-filled by GpSimd during the DMA ramp, so the
            # epilogue chain no longer waits on a 900ns Q7 copy
            aggTs = []
            for t in range(TS):
                at = constp.tile([NKRON + 8, P], bf16, tag=f"aggT{t}")
                nc.gpsimd.tensor_copy(at[NKRON:NKRON + 8, :],
                                      xT8c_sb[:, t * P:(t + 1) * P])
                aggTs.append(at)

            # block -> (tile, b, nb) map for the flat pair loop
            blk2tile = []
            for t in range(TS):
                for b in range(counts[t]):
                    blk2tile.append((t, b, counts[t]))

            def _epilogue(t, acc):
                # ghost edges cover every node slot, so den >= 1 there and
                # exp() never rounds to zero in bf16: no +eps guard needed
                rden = blkp.tile([P, H], f32, tag="rden")
                nc.vector.reciprocal(rden[:], acc[:, 0:8])
                aggs = blkp.tile([P, NKRON], bf16, tag="aggs")
                nc.vector.scalar_tensor_tensor(
                    out=aggs[:].rearrange("p (a b) -> p a b", b=H),
                    in0=acc[:, 8:NU].rearrange("p (a b) -> p a b", b=H),
                    scalar=1.0,
                    in1=rden[:, None, :].to_broadcast([P, NF12, H]),
                    op0=mybir.AluOpType.mult,
                    op1=mybir.AluOpType.mult,
                )
                psT2 = psmiscp.tile([NKRON, P], bf16, tag="T2")
                nc.tensor.transpose(out=psT2[:], in_=aggs[:], identity=identb_sb[:])
                aggT = aggTs[t]
                nc.scalar.copy(aggT[0:NKRON, :], psT2[:])
                pso = psmiscp.tile([P, DIM], f32, tag="T")
                nc.tensor.matmul(pso[:], lhsT=aggT[:], rhs=WblkWo_sb[:],
                                 start=True, stop=True)
                nc.scalar.activation(hall[:, t * DIM:(t + 1) * DIM], pso[:],
                                     mybir.ActivationFunctionType.Relu)
                if t % 4 == 3 or t == TS - 1:
                    s = t // 4
                    c0, c1 = s * 4 * DIM, (t + 1) * DIM
                    nc.tensor.matmul(psS[:, 0:c1 - c0], lhsT=ones_sb[:],
                                     rhs=hall[:, c0:c1],
                                     start=(s == 0), stop=(s == nslice - 1))

            acc_state = [None]

            def scatter_one(item, q):
                g0, rhswm, ohc, cb0 = item
                t, b, nb = blk2tile[g0 + q]
                if b == 0:
                    acc_state[0] = psaccp.tile([P, NU], f32, tag="acc",
                                               name="acc")
                acc = acc_state[0]
                nc.tensor.matmul(acc[:], lhsT=ohc[:, cb0 + q, :],
                                 rhs=rhswm[:, q, :],
                                 start=(b == 0), stop=(b == nb - 1))
                if b == nb - 1:
                    _epilogue(t, acc)

            pend = []
            for g in range(0, C, GB):
                ci = cidx_of[g]
                if g == bounds[ci] and ci not in chunks:
                    load_chunk(ci)
                st, ohc, cg0 = chunks[ci]
                cb = g - cg0

                # interleave this group's main matmuls 1:1 with the scatter
                # matmuls of the group issued two iterations ago: alternating
                # PSUM targets lets the PE pull the next weight-load ahead.
                sc = pend.pop(0) if len(pend) > 2 else None
                psm = psmainp.tile([P, GB, H], f32, tag="main")
                for q in range(GB):
                    nc.tensor.matmul(psm[:, q, :],
                                     lhsT=st[:, (cb + q) * P:(cb + q + 1) * P],
                                     rhs=rhs96_sb[:], start=True, stop=True)
                    if sc is not None:
                        scatter_one(sc, q)

                # payload cols = [w(8) | (j,h)(96)] so the epilogue's rden
                # broadcast is innermost-packed (3D scalar_tensor_tensor)
                rhswm = blkp.tile([P, GB, NU], bf16, tag="rhswm")
                nc.scalar.activation(rhswm[:, :, 0:8], psm[:],
                                     mybir.ActivationFunctionType.Exp)
                se_sb = se12a_sb if g + GB <= SE_SPLIT else se12b_sb
                gg = g if g + GB <= SE_SPLIT else g - SE_SPLIT
                nc.vector.tensor_tensor(
                    out=rhswm[:, :, 8:NU].rearrange("p c (a b) -> p c a b",
                                                    b=H),
                    in0=se_sb[:, gg:gg + GB, :, None].to_broadcast(
                        [P, GB, NF12, H]),
                    in1=rhswm[:, :, None, 0:8].to_broadcast([P, GB, NF12, H]),
                    op=mybir.AluOpType.mult,
                )
                pend.append((g, rhswm, ohc, cb))
            while pend:
                item = pend.pop(0)
                for q in range(GB):
                    scatter_one(item, q)

            Scopy = constp.tile([1, 4 * DIM], f32, tag="Scopy")
            nc.vector.tensor_copy(Scopy[:], psS[:])
            nc.sync.dma_start(out=S_out, in_=Scopy[:])

    nc.compile()
    return nc


# ---------------------------------------------------------------- entry point
def kernel(**inputs):
    pos = np.asarray(inputs["pos"], np.float32)
    edge_attr = np.asarray(inputs["edge_attr"], np.float32)
    emb = np.asarray(inputs["emb"], np.float32)
    Wq = np.asarray(inputs["Wq"], np.float32)
    Wk = np.asarray(inputs["Wk"], np.float32)
    Wv = np.asarray(inputs["Wv"], np.float32)
    We = np.asarray(inputs["We"], np.float32)
    Wo = np.asarray(inputs["Wo"], np.float32)
    lin_w = np.asarray(inputs["lin_w"], np.float32)
    lin_b = np.asarray(inputs["lin_b"], np.float32)
    z = inputs["z"]
    edge_index = inputs["edge_index"]

    shared, percore, meta = _host_prep(pos, edge_attr, emb, Wq, Wk, Wv, We, Wo,
                                       z, edge_index)
    N = meta["N"]

    key = (tuple(meta["counts"]), meta["C"], meta["TS"], meta["npad"],
           meta["SE_SPLIT"])
    nc = _PROG_CACHE.get(key)
    if nc is None:
        nc = _build_program(meta["counts"], meta["C"], meta["TS"],
                            meta["npad"], meta["SE_SPLIT"])
        _PROG_CACHE[key] = nc

    in_maps = []
    for j in range(N_CORES):
        m = {
            "rhs96": shared["rhs96"],
            "WblkWo": shared["WblkWo"],
            "ident": shared["ident"],
            "ones": shared["ones"],
            "srcfacT": percore["srcfacT"][j],
            "se12a": percore["se12a"][j],
            "se12b": percore["se12b"][j],
            "ohmat": percore["ohmat"][j],
            "xT8c": percore["xT8c"][j],
        }
        in_maps.append(m)

    from concourse.bass_utils import run_bass_kernel_spmd
    res = run_bass_kernel_spmd(nc, in_maps, core_ids=list(range(N_CORES)),
                               trace=PROFILE, trace_cores=TRACE_CORES)
    global LAST_RESULT
    LAST_RESULT = res
    S = np.zeros(DIM, np.float64)
    for r in res.results:
        S += r["S_out"][0].astype(np.float64).reshape(4, DIM).sum(axis=0)
    y = (S.astype(np.float32) @ lin_w) / np.float32(N) + lin_b
    return y.reshape(1, 1).astype(np.float32)
